# revision 39
# baseline (speedup 1.0000x reference)
"""MoE top-1 feed-forward (DeepSpeed-style) on 8 Trainium2 NeuronCores.

Strategy (expert parallelism, per the sharding hint):
  - Host computes the (tiny) gate: logits = x @ Wg, softmax, top-1 expert id
    and gate prob per token (float64 for a faithful argmax).
  - Core e holds W1[e]/b1[e]/W2[e]; tokens routed to expert e are dispatched
    to core e, padded to a fixed capacity C=256 so all 8 cores run one SPMD
    program.  Tokens beyond capacity (~2% for the target batch) are computed
    exactly on the host (standard capacity-limited MoE dispatch, but with a
    host fixup instead of drops so the result is exact).
  - Each core runs the dense FFN for its tokens with tokens on the moving
    (free) dimension so no transposes are needed anywhere:
        hT = silu(W1^T @ xT + b1);  yT = W2^T @ hT
  - Weights are packed as flat 128x128 blocks in PE consumption order and
    streamed over the three DMA-initiating rings (SP / ACT / Pool) with a
    greedy earliest-completion schedule against measured queue rates, so
    every tile lands just before the PE consumes it.
  - The PE is kept busy from ~3.5us with warmup matmuls on a scratch tile:
    the HAM clock gate unthrottles (K=8/8, 2.4 GHz) after ~3.4us of
    sustained activity, so the real matmuls start warm instead of paying
    the 1.2 GHz cold ramp.  ACT activation tables are preloaded the same
    way (dummy silu/copy) during the DMA dead time.
  - mm2 trails mm1 by DEFER chunks (PSUM-resident y accumulators) to ride
    out W2 arrival jitter; the tail interleaves PSUM evacuation (DVE+ACT,
    casting to bf16) with the final matmuls and streams y out over three
    rings as soon as each slice is ready.
  - Host combines: out[token] = gate * (y + b2[expert]).
"""

import os
import sys

import numpy as np

try:
    import concourse.mybir as mybir  # noqa: F401
except ModuleNotFoundError:  # fallback if the site hooks aren't installed
    sys.path.insert(0, "/opt/trn_rl_repo")

import concourse.mybir as mybir
import concourse.tile as tile
from concourse import bacc
from concourse.bass_utils import run_bass_kernel_spmd

N_CORES = 8

# Token capacity per core. 256 balances PE time against the ~30us weight
# stream; overflow tokens (2% at the target batch) are fixed up on host.
CAP = int(os.environ.get("BASS_MOE_C", "256"))
DEFER = int(os.environ.get("BASS_MOE_DEFER", "6"))  # mm2 lag in f-chunks
NWARM = int(os.environ.get("BASS_MOE_NWARM", "12"))  # PE warmup matmuls (N=512)

_CACHE: dict = {}


def _block_groups(nblocks, kd):
    """W1 host-packing groups: [kd, 2kd] raw pre-barrier images (blocks
    0..3kd-1 on the ACT/Pool rings), then a 1-chunk lead and uniform
    2-chunk segments for the Tile-world stream."""
    groups = [kd, 2 * kd, kd]
    rem = nblocks - 4 * kd
    while rem > 0:
        w = min(2 * kd, rem)
        groups.append(w)
        rem -= w
    return groups


def _block_groups2(nblocks, kd):
    """W2 segments: uniform 2-chunk tiles."""
    groups = []
    rem = nblocks
    while rem > 0:
        w = min(2 * kd, rem)
        groups.append(w)
        rem -= w
    return groups


def _schedule(w1_groups, w2_groups, chunk_us, t0_us, kd):
    """Greedy earliest-finish assignment of weight tiles to the three DMA
    rings. Returns ({queue: [(kind, tile_idx), ...]}, act_mid) where
    act_mid[i] = f-chunk after whose silu the i-th ACT tile is issued.

    Rates (GB/s == KB/us) and ring-start times are HW-measured.  Pinned
    heads (the PE-start critical path): xa+xb on sync, W1 tile0 on act,
    W1 tile1 + b1 on pool.  The ACT engine must stay free for the silus
    from ~13.5us on, and its HWDGE ring stalls the engine if more than
    ~3 big DMAs are outstanding - so ACT's remaining tiles are issued one
    at a time from inside the chunk loop (act_mid), which also bounds its
    ring depth.
    """
    # x, b1 and W1 blocks [0, 3kd) are raw pre-barrier loads (land ~4us);
    # the Tile-world stream starts post-barrier.
    q = {
        "sync": {"clock": 9.0, "rate": 105.0},
        "act": {"clock": 9.4, "rate": 105.0},
        "pool": {"clock": 10.35, "rate": 135.0},
    }
    # The ACT HWDGE ring fits ~3 outstanding DMAs and a full ring stalls
    # the engine (and so the silus): cap ACT at 3 tiles, all up-front.
    act_cap = 3
    act_n = 0

    items = []  # (deadline_us, size_kb, kind, idx)
    o = 0
    for i, g in enumerate(w1_groups):
        if i > 1:  # groups 0/1 are the raw pre-barrier images
            items.append((t0_us + chunk_us * (o // kd), g * 32.0, "g", i))
        o += g
    o = 0
    for i, g in enumerate(w2_groups):
        items.append((t0_us + chunk_us * (o // kd + DEFER), g * 32.0, "p", i))
        o += g
    items.sort(key=lambda it: it[0])

    sched = {"sync": [], "act": [], "pool": []}
    act_mid = []
    report = []
    for dl, kb, kind, idx in items:
        best, best_t = None, None
        for name, st in q.items():
            if name == "act" and act_n >= act_cap:
                continue
            t = st["clock"] + kb / st["rate"]
            if best_t is None or t < best_t:
                best, best_t = name, t
        q[best]["clock"] = best_t
        sched[best].append((kind, idx))
        if best == "act":
            act_n += 1
        report.append((kind, idx, best, round(best_t, 1), round(dl, 1)))
    if os.environ.get("BASS_MOE_DEBUG"):
        for r in report:
            slack = r[4] - r[3]
            print(f"  {r[0]}{r[1]:<3d} -> {r[2]:5s} eta={r[3]:5.1f} dl={r[4]:5.1f} "
                  f"slack={slack:+.1f}{'  LATE' if slack < 0 else ''}")
        print({k: v for k, v in q.items()})
    return sched, act_mid


def _build_bass(C, D, F):
    f32 = mybir.dt.float32
    dt_io = mybir.dt.bfloat16

    KD, KF = D // 128, F // 128
    NB = KD * KF  # 128x128 blocks per weight matrix
    GR1 = _block_groups(NB, KD)
    GR2 = _block_groups2(NB, KD)
    assert 224 <= C <= 512 and C % 2 == 0

    # block -> (tile idx, offset within tile), per weight matrix
    def block_map(groups):
        m, t, off = {}, 0, 0
        o = 0
        for t, g in enumerate(groups):
            for j in range(g):
                m[o + j] = (t, j)
            o += g
        return m

    bm1, bm2 = block_map(GR1), block_map(GR2)

    nc = bacc.Bacc(None, target_bir_lowering=False, debug=False)
    # Host-packed images (see kernel() for the packing):
    #   xA/xB [128, 3*C]      col d*C+t = x^T[d*128+p, t], d in 0..2 / 3..5
    #   w1    [NB*128*128]    flat tiles; tile t = blocks b=f*KD+d in
    #                         consumption order, [128, g*128] partition-major
    #   w2    [NB*128*128]    same layout, blocks b=f*KD+dd
    #   b1r   [128, KF]       b1[f*128+p] at [p, f]
    #   yA/yB/yC [128, 2*C]   output yT d-blocks (0,1) / (2,3) / (4,5)
    xA = nc.dram_tensor("xA", [128, 3 * C], dt_io, kind="ExternalInput")
    xB = nc.dram_tensor("xB", [128, 3 * C], dt_io, kind="ExternalInput")
    w1 = nc.dram_tensor("w1", [NB * 128 * 128], dt_io, kind="ExternalInput")
    w2 = nc.dram_tensor("w2", [NB * 128 * 128], dt_io, kind="ExternalInput")
    b1r = nc.dram_tensor("b1r", [128, KF], f32, kind="ExternalInput")
    yA = nc.dram_tensor("yA", [128, 2 * C], dt_io, kind="ExternalOutput")
    yB = nc.dram_tensor("yB", [128, 2 * C], dt_io, kind="ExternalOutput")
    yC = nc.dram_tensor("yC", [128, 2 * C], dt_io, kind="ExternalOutput")

    silu = mybir.ActivationFunctionType.Silu

    # PE pace: ~(C/2.4 + 2.5)ns per matmul, 12 matmuls per f-chunk
    chunk_us = 2 * KD * (C / 2.4 + 2.5) / 1000.0
    sched, act_mid = _schedule(GR1, GR2, chunk_us, 8.0, KD)

    # ---- pre-barrier raw loads -------------------------------------
    # The Tile preamble holds every engine at an all-engine barrier until
    # ~7.2us (slowest-engine boot + instruction-queue priming), and only
    # then do Tile-tracked DMA issues start (first bytes ~8.9us).  The
    # PE-start critical data (x, b1, W1 f-chunks 0..2) is instead loaded
    # by raw bass DMAs emitted BEFORE the TileContext: they issue at
    # engine boot (~0.3us) and land during the boot barrier.  One shared
    # semaphore counts completions (6 DMAs x 16 engine-increments); the
    # first real matmul waits for all of it and everything downstream is
    # ordered transitively.
    s_raw = nc.alloc_semaphore("raw_in")
    xa_r = nc.alloc_sbuf_tensor("xa_r", [128, 3 * C], dt_io)
    xb_r = nc.alloc_sbuf_tensor("xb_r", [128, 3 * C], dt_io)
    b1_r = nc.alloc_sbuf_tensor("b1_r", [128, KF], f32)
    wr_a = nc.alloc_sbuf_tensor("wr_a", [128, KD * 128], dt_io)  # blocks [0, KD)
    wr_p = nc.alloc_sbuf_tensor("wr_p", [128, 2 * KD * 128], dt_io)  # [KD, 3KD)
    nc.sync.dma_start(out=xa_r[:], in_=xA[:]).then_inc(s_raw, 16)
    nc.sync.dma_start(out=xb_r[:], in_=xB[:]).then_inc(s_raw, 16)
    nc.scalar.dma_start(out=b1_r[:], in_=b1r[:]).then_inc(s_raw, 16)
    nc.scalar.dma_start(
        out=wr_a[:],
        in_=w1[0 : KD * 128 * 128].rearrange("(p w) -> p w", p=128),
    ).then_inc(s_raw, 16)
    nc.gpsimd.dma_start(
        out=wr_p[:],
        in_=w1[KD * 128 * 128 : 3 * KD * 128 * 128].rearrange("(p w) -> p w", p=128),
    ).then_inc(s_raw, 16)
    RAW_DONE = 5 * 16  # five raw DMAs x 16 SDMA-engine increments
    # Gate the raw-data consumers (PE: x + lead W1, ACT: b1) here, BEFORE
    # the TileContext: engine program order then protects every in-context
    # instruction, the waits resolve (~4us) before the boot barrier exits
    # (~7.3us), and the Tile scheduler's simulation never sees the foreign
    # semaphore (an in-context wait on it deadlocks the sim).
    nc.tensor.wait_ge(s_raw, RAW_DONE)
    nc.scalar.wait_ge(s_raw, RAW_DONE)

    with tile.TileContext(nc) as tc:
        with (
            tc.tile_pool(name="sp", bufs=1) as sp,  # static: x, weights, b1, y
            tc.tile_pool(name="hp", bufs=8) as hp,
            tc.tile_pool(name="ps_h", bufs=2, space="PSUM") as ps_h,
            tc.tile_pool(name="ps_y", bufs=1, space="PSUM") as ps_y,
        ):
            # ---- tiles ----
            wsc = sp.tile([128, 4], f32, tag="wsc", name="wsc")
            wsb = sp.tile([128, 4], dt_io, tag="wsb", name="wsb")
            w1t = [
                sp.tile([128, g * 128], dt_io, tag=f"w1_{t}", name=f"w1t{t}")
                if t >= 2
                else None
                for t, g in enumerate(GR1)
            ]
            w2t = [
                sp.tile([128, g * 128], dt_io, tag=f"w2_{t}", name=f"w2t{t}")
                for t, g in enumerate(GR2)
            ]
            yt = sp.tile([128, KD * C], dt_io, tag="yt", name="yt")
            py = [
                ps_y.tile([128, C], f32, tag=f"y{dd}", name=f"py{dd}")
                for dd in range(KD)
            ]

            w1_offs, w2_offs = [], []
            o = 0
            for g in GR1:
                w1_offs.append(o)
                o += g
            o = 0
            for g in GR2:
                w2_offs.append(o)
                o += g

            def load_w(eng, kind, t):
                src, tiles, offs, grs = (
                    (w1, w1t, w1_offs, GR1) if kind == "g" else (w2, w2t, w2_offs, GR2)
                )
                o = offs[t] * 128 * 128
                n = grs[t] * 128 * 128
                eng.dma_start(
                    out=tiles[t][:],
                    in_=src[o : o + n].rearrange("(p w) -> p w", p=128),
                )

            nc.vector.memset(wsc[:], 0.0)

            # ---- DMA issue blocks (per-engine program order == ring order)
            for kind, t in sched["sync"]:
                load_w(nc.sync, kind, t)
            # ACT: Silu table preload (dtypes exactly matching the real silu
            # so no table miss hits the critical path later), then its few
            # ring-safe weight tiles.
            nc.scalar.activation(
                wsb[:, 0:1], wsc[:, 0:1], silu, bias=wsc[:, 1:2]
            )
            for kind, t in sched["act"]:
                load_w(nc.scalar, kind, t)
            for kind, t in sched["pool"]:
                load_w(nc.gpsimd, kind, t)

            def xsl(d):
                return (
                    xa_r[:, d * C : (d + 1) * C]
                    if d < 3
                    else xb_r[:, (d - 3) * C : (d - 2) * C]
                )

            def w1sl(f, d):
                b = f * KD + d
                if b < KD:
                    return wr_a[:, b * 128 : (b + 1) * 128]
                if b < 3 * KD:
                    return wr_p[:, (b - KD) * 128 : (b - KD + 1) * 128]
                t, j = bm1[b]
                return w1t[t][:, j * 128 : (j + 1) * 128]

            def w2sl(f, dd):
                t, j = bm2[f * KD + dd]
                return w2t[t][:, j * 128 : (j + 1) * 128]

            def emit_mm2(f, ht, last=False):
                for dd in range(KD):
                    nc.tensor.matmul(
                        py[dd][:],
                        w2sl(f, dd),
                        ht[:],
                        start=(f == 0),
                        stop=(f == KF - 1),
                    )
                    if last:
                        # stagger PSUM evacuation (all DVE - keeps ACT free
                        # and avoids the Copy activation-table load entirely)
                        dst = yt[:, dd * C : (dd + 1) * C]
                        nc.vector.tensor_copy(dst, py[dd][:])
                        if dd == 1:
                            nc.sync.dma_start(out=yA[:], in_=yt[:, 0 : 2 * C])
                        elif dd == 3:
                            nc.gpsimd.dma_start(out=yB[:], in_=yt[:, 2 * C : 4 * C])
                        elif dd == 5:
                            nc.scalar.dma_start(out=yC[:], in_=yt[:, 4 * C : 6 * C])

            pend: list = []
            for f in range(KF):
                ph = ps_h.tile([128, C], f32, tag="hps", name="ph")
                for d in range(KD):
                    nc.tensor.matmul(
                        ph[:], w1sl(f, d), xsl(d), start=(d == 0), stop=(d == KD - 1)
                    )
                ht = hp.tile([128, C], dt_io, tag="ht", name="ht")
                nc.scalar.activation(ht[:], ph[:], silu, bias=b1_r[:, f : f + 1])
                pend.append((f, ht))
                # mm2 trails mm1 by DEFER chunks mid-stream (rides out W2
                # arrival jitter), draining to ~2 near the end so the last
                # mm1 isn't followed by a long pure-mm2 tail.
                target = DEFER if f < KF - DEFER + 1 else max(2, KF - 1 - f)
                while len(pend) > target:
                    emit_mm2(*pend.pop(0))
            while pend:
                f, ht = pend.pop(0)
                emit_mm2(f, ht, last=(f == KF - 1))

    # Reset the raw-load semaphore (Tile's epilogue only clears its own) so
    # repeated executions of the NEFF start from zero.
    nc.gpsimd.dma_reset(range(s_raw.num, s_raw.num + 1))
    nc.gpsimd.sem_clear(range(s_raw.num, s_raw.num + 1))

    nc.compile()
    return nc


def _get_bass(C, D, F):
    key = (C, D, F, DEFER, NWARM)
    if key not in _CACHE:
        _CACHE[key] = _build_bass(C, D, F)
    return _CACHE[key]


def _gate_host(x, Wg):
    """Top-1 gating in float64: returns (expert_idx [T], gate [T] f32)."""
    logits = x.astype(np.float64) @ Wg.astype(np.float64)
    m = logits.max(-1, keepdims=True)
    p = np.exp(logits - m)
    p /= p.sum(-1, keepdims=True)
    return p.argmax(-1), p.max(-1).astype(np.float32)


def _ffn_host(x, W1e, b1e, W2e, b2e):
    h = x @ W1e + b1e
    h = h * (1.0 / (1.0 + np.exp(-h)))
    return h @ W2e + b2e


def _kernel_numpy(x, Wg, W1, b1, W2, b2):
    """Reference-equivalent fallback (host only)."""
    idx, gate = _gate_host(x, Wg)
    out = np.zeros_like(x)
    for e in range(W1.shape[0]):
        ids = np.nonzero(idx == e)[0]
        if ids.size == 0:
            continue
        out[ids] = gate[ids, None] * _ffn_host(x[ids], W1[e], b1[e], W2[e], b2[e])
    return out


def kernel(hidden_states, Wg, W1, b1, W2, b2):
    hidden_states = np.asarray(hidden_states)
    Wg = np.asarray(Wg, dtype=np.float32)
    W1 = np.asarray(W1, dtype=np.float32)
    b1 = np.asarray(b1, dtype=np.float32)
    W2 = np.asarray(W2, dtype=np.float32)
    b2 = np.asarray(b2, dtype=np.float32)

    orig_shape = hidden_states.shape
    D = orig_shape[-1]
    x = np.ascontiguousarray(hidden_states, dtype=np.float32).reshape(-1, D)
    E, _, F = W1.shape
    KD, KF = D // 128, F // 128

    if E != N_CORES or D % 128 != 0 or F % 128 != 0:
        return _kernel_numpy(x, Wg, W1, b1, W2, b2).reshape(orig_shape)

    C = CAP
    idx, gate = _gate_host(x, Wg)
    order = np.argsort(idx, kind="stable")
    counts = np.bincount(idx, minlength=E)
    starts = np.concatenate([[0], np.cumsum(counts)])

    import ml_dtypes

    np_io = ml_dtypes.bfloat16
    nc = _get_bass(C, D, F)

    GR1 = _block_groups(KD * KF, KD)
    GR2 = _block_groups2(KD * KF, KD)

    def pack_blocks(mat, groups, f_major_rows):
        # mat: [D, F] (W1, block b=f*KD+d) or [F, D] (W2, block b=f*KD+dd)
        if f_major_rows:
            blk = mat.reshape(KF, 128, KD, 128)  # [f, p, dd, c]
            blks = blk.transpose(0, 2, 1, 3).reshape(KD * KF, 128, 128)
        else:
            blk = mat.reshape(KD, 128, KF, 128)  # [d, p, f, c]
            blks = blk.transpose(2, 0, 1, 3).reshape(KD * KF, 128, 128)
        parts = []
        o = 0
        for g in groups:
            t = blks[o : o + g]  # [g, 128, 128]
            parts.append(t.transpose(1, 0, 2).reshape(-1))  # [128, g*128] flat
            o += g
        return np.concatenate(parts)

    in_maps = []
    keep_ids, over_ids = [], []
    for e in range(E):
        ids = order[starts[e] : starts[e + 1]]
        keep = ids[:C]
        keep_ids.append(keep)
        over_ids.append(ids[C:])
        xe = np.zeros((C, D), dtype=np.float32)
        xe[: keep.size] = x[keep]
        xTr = xe.reshape(C, KD, 128).transpose(2, 1, 0).reshape(128, KD * C)
        in_maps.append(
            {
                "xA": np.ascontiguousarray(xTr[:, : 3 * C]).astype(np_io, copy=False),
                "xB": np.ascontiguousarray(xTr[:, 3 * C :]).astype(np_io, copy=False),
                "w1": pack_blocks(W1[e], GR1, False).astype(np_io, copy=False),
                "w2": pack_blocks(W2[e], GR2, True).astype(np_io, copy=False),
                "b1r": np.ascontiguousarray(b1[e].reshape(KF, 128).T),
            }
        )

    res = run_bass_kernel_spmd(nc, in_maps, list(range(N_CORES)))

    out = np.zeros_like(x)
    for e in range(E):
        keep = keep_ids[e]
        if keep.size:
            yr = np.concatenate(
                [
                    np.asarray(res.results[e]["yA"], dtype=np.float32),
                    np.asarray(res.results[e]["yB"], dtype=np.float32),
                    np.asarray(res.results[e]["yC"], dtype=np.float32),
                ],
                axis=1,
            )  # [128, KD*C]
            y = yr.reshape(128, KD, C).transpose(2, 1, 0).reshape(C, D)[: keep.size]
            out[keep] = gate[keep, None] * (y + b2[e])
        ov = over_ids[e]
        if ov.size:
            out[ov] = gate[ov, None] * _ffn_host(x[ov], W1[e], b1[e], W2[e], b2[e])
    return out.reshape(orig_shape)
